# revision 1
# baseline (speedup 1.0000x reference)
"""Trainium2 Bass kernel for nn_ProtoCycleModel (retrieval_knn).

Problem: P=65536 prototypes, C=64 classes, D=256.
Per class c (rows c::64 of each table, n=1024):
    p2_inv = (p2_c - b) @ inv(W.T)          # y-side of direction "source"
    p1_fwd = p1_c @ W.T + b                 # y-side of direction "target"
    loss_src[c] = mean_i min_j ||p1_c[i] - p2_inv[j]||^2
    loss_tgt[c] = mean_i min_j ||p2_c[i] - p1_fwd[j]||^2
Output: (2, 64) fp32.

Sharding: class axis across 8 cores (8 classes/core). Each core:
  - loads its (8*1024, 256) slices of both tables (row-major, contiguous)
  - PE-transposes them to d-major (fp32 exact)
  - computes transformed tables directly in transposed space:
        yT = Mat @ xT + bias   (Mat = -2*inv(W.T)-style, folded scale -2)
    so the pairwise matmul G = xT.T @ yT gives -2 * x.y' directly.
  - |y'|^2 row: ones-matmul over Square(transform psum + bias) (scale 1/4
    baked into a 0.25-constant stationary matrix), broadcast to all 128
    partitions for free via M=128 stationary ones.
  - per i-tile: fused DVE tensor_tensor_reduce: min_j (G + |y'|^2) -> [128,1]
  - per-class scalars via ones-matmul cross-partition sum; host gathers.

All matmuls run in float32r (TF32-like, full PE rate at N>=512, ~16x more
accurate than bf16). Everything else fp32.
"""

import numpy as np

P, C, D = 65536, 64, 256
N_CORES = 8
CPC = C // N_CORES          # classes per core = 8
NPC = P // C                # prototypes per class = 1024
IT = NPC // 128             # i-tiles per class = 8

# ys application mode: "ttr" = fused DVE tensor_tensor_reduce;
# "fold" = K=1 matmul folds ys row into PSUM, then plain tensor_reduce.
YS_MODE = "fold"
import os as _os
PSG_WIDE = _os.environ.get("K_PSG_WIDE", "0") == "1"   # [128,1024] G tiles
PSG_BUFS = int(_os.environ.get("K_PSG_BUFS", "4"))
PSM_BUFS = int(_os.environ.get("K_PSM_BUFS", "2"))

_CACHE = {}


def _build_bass():
    import concourse.bass as bass
    from concourse import bacc
    import concourse.tile as tile
    from concourse import mybir
    from concourse.masks import make_identity

    FP32 = mybir.dt.float32
    FP32R = mybir.dt.float32r
    BF16 = mybir.dt.bfloat16
    AF = mybir.ActivationFunctionType
    ALU = mybir.AluOpType
    AX = mybir.AxisListType

    nc = bacc.Bacc(None, target_bir_lowering=False)

    p1_d = nc.dram_tensor("p1", [CPC * NPC, D], FP32, kind="ExternalInput")
    p2_d = nc.dram_tensor("p2", [CPC * NPC, D], FP32, kind="ExternalInput")
    # mats[dir][kchunk] : [128, 256] fp32, lhsT layout [d, d'] with the -2
    # scale folded in.  dir 0 = source (V2 = -2*inv(W.T)), dir 1 = target
    # (Wt2 = -2*W.T).
    mats_d = nc.dram_tensor("mats", [2, 2, 128, D], FP32, kind="ExternalInput")
    consts_d = nc.dram_tensor("consts", [128, 385], FP32, kind="ExternalInput")
    # biases[dir] : [128, 2] fp32 (column = d' chunk);  dir0 = +2*(b@V),
    # dir1 = -2*b.
    bias_d = nc.dram_tensor("biases", [2, 128, 2], FP32, kind="ExternalInput")
    out_d = nc.dram_tensor("out", [1, 2 * CPC], FP32, kind="ExternalOutput")

    with tile.TileContext(nc) as tc:
        with (
            tc.tile_pool(name="const", bufs=1) as const,
            tc.tile_pool(name="xrow", bufs=6) as xrow_p,
            tc.tile_pool(name="xt", bufs=10) as xt_p,
            tc.tile_pool(name="yt", bufs=8) as yt_p,
            tc.tile_pool(name="sq", bufs=4) as sq_p,
            tc.tile_pool(name="ysb", bufs=4) as ysb_p,
            tc.tile_pool(name="scr", bufs=3) as scr_p,
            tc.tile_pool(name="psg", bufs=PSG_BUFS, space="PSUM") as psg_p,
            tc.tile_pool(name="psm", bufs=PSM_BUFS, space="PSUM") as psm_p,
        ):
            # ---- constants ----
            cblk_raw = const.tile([128, 385], FP32)
            nc.scalar.dma_start(cblk_raw[:], consts_d[:])
            cblk = const.tile([128, 385], FP32R)
            nc.vector.tensor_copy(cblk[:], cblk_raw[:])

            mats_raw = const.tile([128, 2, 2, D], FP32)
            nc.scalar.dma_start(mats_raw[:], mats_d[:].rearrange("a b p d -> p a b d"))
            mats = const.tile([128, 2, 2, D], FP32R)
            nc.vector.tensor_copy(mats[:], mats_raw[:])

            biases = const.tile([128, 2, 2], FP32)  # [p, dir, dchunk]
            nc.scalar.dma_start(biases[:], bias_d[:].rearrange("a p c -> p a c"))
            identr = cblk[:, 0:128]
            identf = cblk_raw[:, 0:128]
            ones1r = cblk[:, 128:129]
            ones_q = cblk[:, 257:385]

            pmin = const.tile([128, 2 * CPC * IT], FP32)   # col = dir*64+c*8+it
            pmin2 = (const.tile([128, 2 * CPC * IT], FP32, name="pmin2")
                     if not PSG_WIDE else pmin)
            pxs = const.tile([128, 2 * CPC * 2], FP32)     # col = dir*16+c*2+dc

            onesrow = cblk[0:1, 128:256]

            # ---- main loop: software-pipelined (prep one class ahead) ----
            state = {}

            def prep(c):
                xts = [[None, None], [None, None]]  # [table][dchunk]
                for t in range(2):
                    src_d = p1_d if t == 0 else p2_d
                    xr = xrow_p.tile([128, IT, D], FP32, tag="xrow", bufs=3)
                    xrr = xrow_p.tile([128, IT, D], FP32R, tag="xrowr", bufs=4)
                    half = NPC // 2
                    for hh in range(2):
                        nc.sync.dma_start(
                            xr[:, hh * (IT // 2):(hh + 1) * (IT // 2), :],
                            src_d[c * NPC + hh * half:
                                  c * NPC + (hh + 1) * half, :].rearrange(
                                "(k p) d -> p k d", p=128),
                        )
                        nc.vector.tensor_copy(
                            xrr[:, hh * (IT // 2):(hh + 1) * (IT // 2), :],
                            xr[:, hh * (IT // 2):(hh + 1) * (IT // 2), :])
                    for dc in range(2):
                        pst = psm_p.tile([128, 1024], FP32R, tag="misc")
                        for k in range(IT):
                            nc.tensor.transpose(
                                pst[:, k * 128:(k + 1) * 128],
                                xrr[:, k, dc * 128:(dc + 1) * 128],
                                identr,
                            )
                        xt_t = xt_p.tile([128, NPC], FP32R, tag="xt")
                        nc.scalar.copy(xt_t[:], pst[:])
                        xts[t][dc] = xt_t
                        # xs partials: sum_i x^2 per d-partition
                        trash = scr_p.tile([128, NPC], BF16, tag="scr")
                        nc.scalar.activation(
                            trash[:], xt_t[:], AF.Square,
                            accum_out=pxs[:, t * 16 + c * 2 + dc:
                                          t * 16 + c * 2 + dc + 1],
                        )

                yts_all = [[], []]
                ysrow_all = [None, None]
                for dr in range(2):
                    ysrc = xts[1 - dr]    # dir0: y from p2; dir1: y from p1
                    sqs = []
                    for dcp in range(2):   # output d' chunk
                        pstf = psm_p.tile([128, 1024], FP32, tag="misc")
                        for dc in range(2):
                            for ih in range(2):
                                nc.tensor.matmul(
                                    pstf[:, ih * 512:(ih + 1) * 512],
                                    mats[:, dr, dc, dcp * 128:(dcp + 1) * 128],
                                    ysrc[dc][:, ih * 512:(ih + 1) * 512],
                                    start=(dc == 0), stop=(dc == 1),
                                )
                        bias_ap = biases[:, dr, dcp:dcp + 1]
                        yt_t = yt_p.tile([128, NPC], FP32R, tag="yt")
                        nc.scalar.activation(
                            yt_t[:], pstf[:], AF.Identity, bias=bias_ap, scale=1.0)
                        sq_t = sq_p.tile([128, NPC], FP32R, tag="sq")
                        nc.scalar.activation(
                            sq_t[:], pstf[:], AF.Square, bias=bias_ap, scale=1.0)
                        yts_all[dr].append(yt_t)
                        sqs.append(sq_t)

                    psy = psm_p.tile([128, 1024], FP32, tag="misc")
                    for jh in range(2):
                        for dcp in range(2):
                            nc.tensor.matmul(
                                psy[0:1, jh * 512:(jh + 1) * 512],
                                ones_q[:, 0:1],
                                sqs[dcp][:, jh * 512:(jh + 1) * 512],
                                start=(dcp == 0), stop=(dcp == 1),
                            )
                    ysrow = ysb_p.tile([1, NPC], FP32R, tag="ysrow")
                    nc.scalar.copy(ysrow[:], psy[0:1, :])
                    ysrow_all[dr] = ysrow
                state[c] = (xts, yts_all, ysrow_all)

            def pairwise(c):
                xts, yts_all, ysrow_all = state.pop(c)
                for dr in range(2):
                    xside = xts[dr]       # dir0: x = p1; dir1: x = p2
                    yts = yts_all[dr]
                    ysrow = ysrow_all[dr]
                    for it in range(IT):
                        col = dr * 64 + c * 8 + it
                        pgs = [psg_p.tile([128, 512], FP32, tag="g",
                                          name=f"pg{jh}")
                               for jh in range(2)]
                        for dc in range(2):          # stationary reused 2x
                            for jh in range(2):
                                nc.tensor.matmul(
                                    pgs[jh][:],
                                    xside[dc][:, it * 128:(it + 1) * 128],
                                    yts[dc][:, jh * 512:(jh + 1) * 512],
                                    start=(dc == 0), stop=False,
                                )
                        for jh in range(2):          # ys fold, ones stationary
                            nc.tensor.matmul(
                                pgs[jh][:],
                                onesrow,
                                ysrow[:, jh * 512:(jh + 1) * 512],
                                start=False, stop=True,
                            )
                        for jh in range(2):
                            dst = pmin if jh == 0 else pmin2
                            nc.vector.tensor_reduce(
                                out=dst[:, col:col + 1], in_=pgs[jh][:],
                                axis=AX.X, op=ALU.min,
                            )

            prep(0)
            for c in range(CPC):
                if c + 1 < CPC:
                    prep(c + 1)
                pairwise(c)

            # ---- finals ----
            if PSG_WIDE:
                pminc = pmin
            else:
                pminc = const.tile([128, 2 * CPC * IT], FP32, name="pminc")
                nc.vector.tensor_tensor(
                    out=pminc[:], in0=pmin[:], in1=pmin2[:], op=ALU.min)
            red_min = const.tile([128, 16], FP32)
            nc.vector.tensor_reduce(
                out=red_min[:], in_=pminc[:].rearrange("p (g k) -> p g k", k=IT),
                axis=AX.X, op=ALU.add)
            red_xs = const.tile([128, 16], FP32)
            nc.vector.tensor_reduce(
                out=red_xs[:], in_=pxs[:].rearrange("p (g k) -> p g k", k=2),
                axis=AX.X, op=ALU.add)
            red = const.tile([128, 16], FP32R)
            nc.vector.tensor_tensor(
                out=red[:], in0=red_min[:], in1=red_xs[:], op=ALU.add)
            psf = psm_p.tile([1, 16], FP32, tag="misc")
            nc.tensor.matmul(psf[:], ones1r, red[:], start=True, stop=True)
            outrow = const.tile([1, 16], FP32)
            nc.scalar.mul(outrow[:], psf[:], 1.0 / NPC)
            nc.sync.dma_start(out_d[:], outrow[:])

    nc.compile()
    return nc


def _get_nc():
    if "nc" not in _CACHE:
        _CACHE["nc"] = _build_bass()
    return _CACHE["nc"]


def kernel(protos1, protos2, W, b, num_classes):
    from concourse.bass_utils import run_bass_kernel_spmd

    nc_classes = int(num_classes)
    assert nc_classes == C and protos1.shape == (P, D)

    protos1 = np.ascontiguousarray(protos1, dtype=np.float32)
    protos2 = np.ascontiguousarray(protos2, dtype=np.float32)
    W = np.asarray(W, dtype=np.float32)
    b = np.asarray(b, dtype=np.float32)

    # host-side tiny prep: inverse + scaled transform matrices
    V = np.linalg.inv(W.T.astype(np.float64)).astype(np.float32)  # (p2-b)@V
    V2 = (-2.0 * V).astype(np.float32)                 # lhsT [d, d'] dir0
    Wt2 = (-2.0 * W.T).astype(np.float32)              # lhsT [d, d'] dir1
    bias0 = (2.0 * (b.astype(np.float64) @ V.astype(np.float64))).astype(np.float32)
    bias1 = (-2.0 * b).astype(np.float32)
    mats = np.stack([
        np.stack([V2[0:128, :], V2[128:256, :]]),
        np.stack([Wt2[0:128, :], Wt2[128:256, :]]),
    ]).astype(np.float32)                               # [2, 2, 128, 256]
    idb = np.eye(128, dtype=np.float32)
    consts = np.concatenate([
        idb,
        np.ones((128, 129), dtype=np.float32),
        np.full((128, 128), 0.25, dtype=np.float32),
    ], axis=1)
    biases = np.stack([
        bias0.reshape(2, 128).T,                        # [128, 2] cols = chunk
        bias1.reshape(2, 128).T,
    ]).astype(np.float32)                               # [2, 128, 2]

    # class-major reordering: (P, D) -> (C, NPC, D)
    p1c = np.ascontiguousarray(protos1.reshape(NPC, C, D).transpose(1, 0, 2))
    p2c = np.ascontiguousarray(protos2.reshape(NPC, C, D).transpose(1, 0, 2))

    in_maps = []
    for core in range(N_CORES):
        sl = slice(core * CPC, (core + 1) * CPC)
        in_maps.append({
            "p1": np.ascontiguousarray(p1c[sl].reshape(CPC * NPC, D)),
            "p2": np.ascontiguousarray(p2c[sl].reshape(CPC * NPC, D)),
            "mats": mats,
            "biases": biases,
            "consts": consts,
        })

    nc = _get_nc()
    res = run_bass_kernel_spmd(nc, in_maps, core_ids=list(range(N_CORES)))
    _CACHE["last_result"] = res

    out = np.zeros((2, C), dtype=np.float32)
    for core in range(N_CORES):
        row = res.results[core]["out"].reshape(2, CPC)
        out[:, core * CPC:(core + 1) * CPC] = row
    return out



# revision 3
# speedup vs baseline: 1.2145x; 1.2145x over previous
"""Trainium2 Bass kernel for nn_ProtoCycleModel (retrieval_knn).

Problem: P=65536 prototypes, C=64 classes, D=256.
Per class c (rows c::64 of each table, n=1024):
    p2_inv = (p2_c - b) @ inv(W.T)          # y-side of direction "source"
    p1_fwd = p1_c @ W.T + b                 # y-side of direction "target"
    loss_src[c] = mean_i min_j ||p1_c[i] - p2_inv[j]||^2
    loss_tgt[c] = mean_i min_j ||p2_c[i] - p1_fwd[j]||^2
Output: (2, 64) fp32.

Sharding: class axis across 8 cores (8 classes/core).

Layout strategy: the host pre-transposes both tables to d-major per class
(xT = [d, n]) and the DRAM tensors are declared float32r, so the kernel
performs no PE transposes and no dtype-cast copies at all. Per class each
core:
  - DMAs xT tiles [128, 2, 1024] for both tables (d-chunk-major, fp32r).
  - computes transformed tables yT = Mat @ xT (+bias via ACT) with the -2
    scale folded into Mat, so the pairwise matmul G = xT.T @ yT gives
    -2 x.y' directly.  All matmuls run in float32r at full PE rate.
  - |y'|^2 row: M=128 ones-matmul (0.25-valued stationary folds the 1/4
    from yT = -2 y') over ACT-squared transform outputs -> psum
    [128, 1024] replicated on all partitions for free; ACT-copies to SBUF.
  - per i-tile: ONE fused custom-DVE op (ADD_MIN_REDUCE, authored via the
    documented dve_ops Spec API) over the full [128, 1024] G psum tile:
    accum = min_j (G + |y'|^2).  This keeps the ys fold off the PE
    entirely and halves the DVE instruction count vs narrow reduces.
  - |x|^2 totals via ACT Square accum_out; per-class scalars via
    ones-matmul cross-partition sum; host gathers.
"""

import numpy as np

P, C, D = 65536, 64, 256
N_CORES = 8
CPC = C // N_CORES          # classes per core = 8
NPC = P // C                # prototypes per class = 1024
IT = NPC // 128             # i-tiles per class = 8

_CACHE = {}


def _get_addmin_op():
    """Author + register the fused (Src0 + Src1) -> min-reduce custom DVE op
    via the documented Spec/DveOp extension API (see
    trainium-docs/custom-instructions/04-custom-dve-api.md).  accum seeds
    from C1 (s1 at the call site)."""
    import concourse.dve_ops as dve_ops
    from concourse.dve_spec import Spec, Src0, Src1, C1, lower, AluOp
    from concourse.dve_uop import DveOpSpec
    from concourse.dve_ops import DveOp, OPS, _SUB_OPCODE_FOR_NAME, CUSTOM_DVE_SPECS

    name = "ADD_MIN_REDUCE_K"
    if name in _SUB_OPCODE_FOR_NAME:
        return next(o for o in OPS if o.name == name)

    def _ref(in0, in1, s0, s1, imm2):
        b = in0.astype(np.float32) + in1.astype(np.float32)
        acc = np.minimum(np.float32(s1), b.min(axis=-1, keepdims=True))
        return b, acc

    spec = Spec(body=Src0 + Src1, accum=AluOp.MIN, accum_init=C1,
                reference=_ref)
    row = max(_SUB_OPCODE_FOR_NAME.values()) + 1
    assert row < 0x20, "no free custom-DVE opcode row"
    _SUB_OPCODE_FOR_NAME[name] = row
    shas = {}
    for ver in ("v3", "v4"):
        try:
            uops = lower(spec, ver=ver)
            shas[ver] = DveOpSpec(name=name, opcode=row, uops=uops,
                                  rd1_en=True).sha(ver)
        except Exception:
            pass
    op = DveOp(name, spec, subdim=False, uops_sha=shas)
    OPS.append(op)
    CUSTOM_DVE_SPECS[name] = spec
    return op


def _build_bass():
    import concourse.bass as bass
    from concourse import bacc
    import concourse.tile as tile
    from concourse import mybir

    FP32 = mybir.dt.float32
    FP32R = mybir.dt.float32r
    BF16 = mybir.dt.bfloat16
    AF = mybir.ActivationFunctionType
    ALU = mybir.AluOpType
    AX = mybir.AxisListType

    addmin = _get_addmin_op()

    nc = bacc.Bacc(None, target_bir_lowering=False)

    # host-pre-transposed tables: [table, class, dchunk, 128, n], fp32r bits
    xt_d = nc.dram_tensor("xt", [2, CPC, 2, 128, NPC], FP32R, kind="ExternalInput")
    # mats[dir][kchunk] : [128, 256] fp32r, lhsT layout [d, d'] with the -2
    # scale folded in.  dir 0 = source (-2*inv(W.T)), dir 1 = (-2*W.T).
    mats_d = nc.dram_tensor("mats", [2, 2, 128, D], FP32R, kind="ExternalInput")
    # consts: [:,0:128] = 0.25 block (ysrow stationary), [:,128:129] = ones
    consts_d = nc.dram_tensor("consts", [128, 129], FP32R, kind="ExternalInput")
    # biases[dir] : [128, 2] fp32 (column = d' chunk);  dir0 = +2*(b@V),
    # dir1 = -2*b.
    bias_d = nc.dram_tensor("biases", [2, 128, 2], FP32, kind="ExternalInput")
    out_d = nc.dram_tensor("out", [1, 2 * CPC], FP32, kind="ExternalOutput")

    with tile.TileContext(nc) as tc:
        with (
            tc.tile_pool(name="const", bufs=1) as const,
            tc.tile_pool(name="xt", bufs=4) as xt_p,
            tc.tile_pool(name="yt", bufs=4) as yt_p,
            tc.tile_pool(name="sq", bufs=3) as sq_p,
            tc.tile_pool(name="ysb", bufs=4) as ysb_p,
            tc.tile_pool(name="tr", bufs=2) as tr_p,
            tc.tile_pool(name="psg", bufs=2, space="PSUM") as psg_p,
            tc.tile_pool(name="psm", bufs=2, space="PSUM") as psm_p,
        ):
            # ---- constants ----
            cblk = const.tile([128, 129], FP32R)
            nc.scalar.dma_start(cblk[:], consts_d[:])
            quarters = cblk[:, 0:128]
            ones1r = cblk[:, 128:129]

            mats = const.tile([128, 2, 2, D], FP32R)
            nc.scalar.dma_start(mats[:], mats_d[:].rearrange("a b p d -> p a b d"))

            biases = const.tile([128, 2, 2], FP32)  # [p, dir, dchunk]
            nc.scalar.dma_start(biases[:], bias_d[:].rearrange("a p c -> p a c"))

            pmin = const.tile([128, 2 * CPC * IT], FP32)   # col = dir*64+c*8+it
            pxs = const.tile([128, 2 * CPC], FP32)         # col = table*8+c
            scratch = const.tile([128, NPC], FP32)         # custom-op elementwise out

            # ---- main loop: software-pipelined (prep one class ahead) ----
            state = {}

            def prep(c):
                xts = []
                for t in range(2):
                    xt = xt_p.tile([128, 2, NPC], FP32R, tag=f"xt{t}")
                    nc.gpsimd.dma_start(
                        xt[:], xt_d[t, c].rearrange("k p n -> p k n"))
                    xts.append(xt)
                    # |x|^2 total accumulator (per d-partition; summed over
                    # partitions in the finals)
                    tr = tr_p.tile([128, 2 * NPC], BF16, tag="tr")
                    nc.scalar.activation(
                        tr[:], xt[:].rearrange("p k n -> p (k n)"), AF.Square,
                        accum_out=pxs[:, t * CPC + c: t * CPC + c + 1],
                    )

                yts = []
                ysbs = []
                for dr in range(2):
                    src = xts[1 - dr]   # dir0: y from p2; dir1: y from p1
                    yt = yt_p.tile([128, 2, NPC], FP32R, tag=f"yt{dr}")
                    sqs = []
                    for dcp in range(2):
                        pstf = psm_p.tile([128, NPC], FP32, tag="m")
                        for dc in range(2):
                            for ih in range(2):
                                nc.tensor.matmul(
                                    pstf[:, ih * 512:(ih + 1) * 512],
                                    mats[:, dr, dc, dcp * 128:(dcp + 1) * 128],
                                    src[:, dc, ih * 512:(ih + 1) * 512],
                                    start=(dc == 0), stop=(dc == 1),
                                )
                        bias_ap = biases[:, dr, dcp:dcp + 1]
                        nc.scalar.activation(
                            yt[:, dcp, :], pstf[:], AF.Identity,
                            bias=bias_ap, scale=1.0)
                        sq = sq_p.tile([128, NPC], FP32R, tag="sq")
                        nc.scalar.activation(
                            sq[:], pstf[:], AF.Square, bias=bias_ap, scale=1.0)
                        sqs.append(sq)

                    # ys row, replicated on all 128 partitions by the M=128
                    # stationary (0.25 entries fold the 1/4 from yt = -2y')
                    psy = psm_p.tile([128, NPC], FP32, tag="m")
                    for jh in range(2):
                        for dcp in range(2):
                            nc.tensor.matmul(
                                psy[:, jh * 512:(jh + 1) * 512],
                                quarters,
                                sqs[dcp][:, jh * 512:(jh + 1) * 512],
                                start=(dcp == 0), stop=(dcp == 1),
                            )
                    ysb = ysb_p.tile([128, NPC], FP32, tag="ysb")
                    nc.scalar.copy(ysb[:], psy[:])
                    ysbs.append(ysb)
                    yts.append(yt)
                state[c] = (xts, yts, ysbs)

            def pairwise(c):
                xts, yts, ysbs = state.pop(c)
                for dr in range(2):
                    xside = xts[dr]       # dir0: x = p1; dir1: x = p2
                    yt = yts[dr]
                    ysb = ysbs[dr]
                    for it in range(IT):
                        col = dr * 64 + c * 8 + it
                        pg = psg_p.tile([128, NPC], FP32, tag="g")
                        for dc in range(2):
                            for jh in range(2):
                                nc.tensor.matmul(
                                    pg[:, jh * 512:(jh + 1) * 512],
                                    xside[:, dc, it * 128:(it + 1) * 128],
                                    yt[:, dc, jh * 512:(jh + 1) * 512],
                                    start=(dc == 0), stop=(dc == 1),
                                )
                        nc.vector._custom_dve(
                            addmin, out=scratch[:], in0=pg[:], in1=ysb[:],
                            s1=3.0e38, accum_out=pmin[:, col:col + 1],
                        )

            prep(0)
            for c in range(CPC):
                if c + 1 < CPC:
                    prep(c + 1)
                pairwise(c)

            # ---- finals ----
            red_min = const.tile([128, 16], FP32)
            nc.vector.tensor_reduce(
                out=red_min[:], in_=pmin[:].rearrange("p (g k) -> p g k", k=IT),
                axis=AX.X, op=ALU.add)
            red = const.tile([128, 16], FP32R)
            nc.vector.tensor_tensor(
                out=red[:], in0=red_min[:], in1=pxs[:], op=ALU.add)
            psf = psm_p.tile([1, 16], FP32, tag="m")
            nc.tensor.matmul(psf[:], ones1r, red[:], start=True, stop=True)
            outrow = const.tile([1, 16], FP32)
            nc.scalar.mul(outrow[:], psf[:], 1.0 / NPC)
            nc.sync.dma_start(out_d[:], outrow[:])

    nc.compile()
    return nc


def _get_nc():
    if "nc" not in _CACHE:
        _CACHE["nc"] = _build_bass()
    return _CACHE["nc"]


def kernel(protos1, protos2, W, b, num_classes):
    from concourse.bass_utils import run_bass_kernel_spmd

    nc_classes = int(num_classes)
    assert nc_classes == C and protos1.shape == (P, D)

    protos1 = np.ascontiguousarray(protos1, dtype=np.float32)
    protos2 = np.ascontiguousarray(protos2, dtype=np.float32)
    W = np.asarray(W, dtype=np.float32)
    b = np.asarray(b, dtype=np.float32)

    # host-side tiny prep: inverse + scaled transform matrices
    V = np.linalg.inv(W.T.astype(np.float64)).astype(np.float32)  # (p2-b)@V
    V2 = (-2.0 * V).astype(np.float32)                 # lhsT [d, d'] dir0
    Wt2 = (-2.0 * W.T).astype(np.float32)              # lhsT [d, d'] dir1
    bias0 = (2.0 * (b.astype(np.float64) @ V.astype(np.float64))).astype(np.float32)
    bias1 = (-2.0 * b).astype(np.float32)
    mats = np.stack([
        np.stack([V2[0:128, :], V2[128:256, :]]),
        np.stack([Wt2[0:128, :], Wt2[128:256, :]]),
    ]).astype(np.float32)                               # [2, 2, 128, 256]
    consts = np.concatenate([
        np.full((128, 128), 0.25, dtype=np.float32),
        np.ones((128, 1), dtype=np.float32),
    ], axis=1)
    biases = np.stack([
        bias0.reshape(2, 128).T,                        # [128, 2] cols = chunk
        bias1.reshape(2, 128).T,
    ]).astype(np.float32)                               # [2, 128, 2]

    # class-major + d-major reordering: (P, D) -> (C, D, NPC) -> chunked
    full = np.stack([protos1, protos2]).reshape(2, NPC, C, D)
    xt_all = np.ascontiguousarray(full.transpose(0, 2, 3, 1))   # [2, C, D, NPC]
    xt_all = xt_all.reshape(2, C, 2, 128, NPC)

    in_maps = []
    for core in range(N_CORES):
        sl = slice(core * CPC, (core + 1) * CPC)
        in_maps.append({
            "xt": np.ascontiguousarray(xt_all[:, sl]),
            "mats": mats,
            "biases": biases,
            "consts": consts,
        })

    nc = _get_nc()
    res = run_bass_kernel_spmd(nc, in_maps, core_ids=list(range(N_CORES)))
    _CACHE["last_result"] = res

    out = np.zeros((2, C), dtype=np.float32)
    for core in range(N_CORES):
        row = res.results[core]["out"].reshape(2, CPC)
        out[:, core * CPC:(core + 1) * CPC] = row
    return out


# revision 23
# speedup vs baseline: 1.3257x; 1.0916x over previous
"""Trainium2 Bass kernel for nn_ProtoCycleModel (retrieval_knn).

Problem: P=65536 prototypes, C=64 classes, D=256.
Per class c (rows c::64 of each table, n=1024):
    p2_inv = (p2_c - b) @ inv(W.T)          # y-side of direction "source"
    p1_fwd = p1_c @ W.T + b                 # y-side of direction "target"
    loss_src[c] = mean_i min_j ||p1_c[i] - p2_inv[j]||^2
    loss_tgt[c] = mean_i min_j ||p2_c[i] - p1_fwd[j]||^2
Output: (2, 64) fp32.

Sharding: class axis across 8 cores (8 classes/core).

Layout: host pre-transposes both tables to d-major per class (xT = [d, n]),
declared float32r in DRAM, so the kernel performs no PE transposes and no
dtype-cast copies.  Per class-dir the core computes the transformed table
yT = Mat @ xT (+bias via ACT, -2 folded into Mat), an |y'|^2 row replicated
over all partitions by an M=128 0.25-valued ones-matmul, then per i-tile a
[128, 1024] G psum tile (2 fp32r matmuls) consumed by ONE fused custom-DVE
op (ADD_MIN_REDUCE, authored via the documented dve_ops Spec API):
accum = min_j (G + |y'|^2).

Scheduling: the instruction stream is emitted as 16 dir-blocks of 8 i-tile
units each; the PE work of the NEXT dir's prep (transform + ysrow matmuls)
is interleaved into the current dir's pairwise slots so the PE never sits
between pairwise phases and the DVE (the pacing engine at ~1227 ns/i-tile)
never starves at class boundaries.  DMAs are issued a full dir-block ahead,
y-side table first, in halves.
"""

import numpy as np

P, C, D = 65536, 64, 256
N_CORES = 8
CPC = C // N_CORES          # classes per core = 8
NPC = P // C                # prototypes per class = 1024
IT = NPC // 128             # i-tiles per class = 8

_CACHE = {}


def _get_addmin_op():
    """Author + register the fused (Src0 + Src1) -> min-reduce custom DVE op
    via the documented Spec/DveOp extension API (see
    trainium-docs/custom-instructions/04-custom-dve-api.md).  accum seeds
    from C1 (s1 at the call site)."""
    import concourse.dve_ops as dve_ops
    from concourse.dve_spec import Spec, Src0, Src1, C1, lower, AluOp
    from concourse.dve_uop import DveOpSpec
    from concourse.dve_ops import DveOp, OPS, _SUB_OPCODE_FOR_NAME, CUSTOM_DVE_SPECS

    name = "ADD_MIN_REDUCE_K"
    if name in _SUB_OPCODE_FOR_NAME:
        return next(o for o in OPS if o.name == name)

    def _ref(in0, in1, s0, s1, imm2):
        b = in0.astype(np.float32) + in1.astype(np.float32)
        acc = np.minimum(np.float32(s1), b.min(axis=-1, keepdims=True))
        return b, acc

    spec = Spec(body=Src0 + Src1, accum=AluOp.MIN, accum_init=C1,
                reference=_ref)
    row = max(_SUB_OPCODE_FOR_NAME.values()) + 1
    assert row < 0x20, "no free custom-DVE opcode row"
    _SUB_OPCODE_FOR_NAME[name] = row
    shas = {}
    for ver in ("v3", "v4"):
        try:
            uops = lower(spec, ver=ver)
            shas[ver] = DveOpSpec(name=name, opcode=row, uops=uops,
                                  rd1_en=True).sha(ver)
        except Exception:
            pass
    op = DveOp(name, spec, subdim=False, uops_sha=shas)
    OPS.append(op)
    CUSTOM_DVE_SPECS[name] = spec
    return op


def _build_bass():
    import concourse.bass as bass
    from concourse import bacc
    import concourse.tile as tile
    from concourse import mybir

    FP32 = mybir.dt.float32
    FP32R = mybir.dt.float32r
    BF16 = mybir.dt.bfloat16
    AF = mybir.ActivationFunctionType
    ALU = mybir.AluOpType
    AX = mybir.AxisListType

    addmin = _get_addmin_op()

    nc = bacc.Bacc(None, target_bir_lowering=False)

    # host-pre-transposed tables: [table, class, dchunk, 128, n], fp32r bits
    xt_d = nc.dram_tensor("xt", [2, CPC, 2, 128, NPC], FP32R, kind="ExternalInput")
    # mats[dir][kchunk] : [128, 256] fp32r, lhsT layout [d, d'] with the -2
    # scale folded in.  dir 0 = source (-2*inv(W.T)), dir 1 = (-2*W.T).
    mats_d = nc.dram_tensor("mats", [2, 2, 128, D], FP32R, kind="ExternalInput")
    # consts: [:,0:128] = 0.25 block (ysrow stationary), [:,128:129] = ones
    consts_d = nc.dram_tensor("consts", [128, 129], FP32R, kind="ExternalInput")
    # biases[dir] : [128, 2] fp32 (column = d' chunk);  dir0 = +2*(b@V),
    # dir1 = -2*b.
    bias_d = nc.dram_tensor("biases", [2, 128, 2], FP32, kind="ExternalInput")
    out_d = nc.dram_tensor("out", [1, 2 * CPC], FP32, kind="ExternalOutput")

    with tile.TileContext(nc) as tc:
        with (
            tc.tile_pool(name="const", bufs=1) as const,
            tc.tile_pool(name="xt", bufs=3) as xt_p,
            tc.tile_pool(name="yt", bufs=2) as yt_p,
            tc.tile_pool(name="sq", bufs=4) as sq_p,
            tc.tile_pool(name="ysb", bufs=3) as ysb_p,
            tc.tile_pool(name="tr", bufs=1) as tr_p,
            tc.tile_pool(name="psg", bufs=2, space="PSUM") as psg_p,
            tc.tile_pool(name="psm", bufs=2, space="PSUM") as psm_p,
        ):
            # ---- constants ----
            cblk = const.tile([128, 129], FP32R)
            nc.sync.dma_start(cblk[:], consts_d[:])
            quarters = cblk[:, 0:128]
            ones1r = cblk[:, 128:129]

            mats = const.tile([128, 2, 2, D], FP32R)
            nc.sync.dma_start(mats[:], mats_d[:].rearrange("a b p d -> p a b d"))

            biases = const.tile([128, 2, 2], FP32)  # [p, dir, dchunk]
            nc.sync.dma_start(biases[:], bias_d[:].rearrange("a p c -> p a c"))

            pmin = const.tile([128, 2 * CPC * IT], FP32)   # col = dir*64+c*8+it
            pxs = const.tile([128, 2 * 2 * CPC], FP32)     # col = k*16+table*8+c
            scratch = const.tile([128, NPC], FP32)         # custom-op elementwise out

            # PE p-state warmup: ramp the clock to full during the first
            # table DMA so the first transform runs at 2.4 GHz.  Reads only
            # the tiny consts block (the first DMA to land).
            warm = psm_p.tile([128, NPC], FP32, tag="m")
            for w in range(14):
                nc.tensor.matmul(warm[:, (w % 4) * 128:(w % 4) * 128 + 128],
                                 quarters, quarters,
                                 start=True, stop=True)

            xts_c = {}      # class -> [xt_table0, xt_table1]
            yts = {}        # (class, dir) -> yt
            ysbs = {}       # (class, dir) -> ysb

            def dma_class(c):
                pair = [None, None]
                for t in (1, 0):            # y-side of dir0 (p2) first
                    xt = xt_p.tile([128, 2, NPC], FP32R, tag=f"xt{t}")
                    for h in range(2):
                        nc.sync.dma_start(
                            xt[:, :, h * 512:(h + 1) * 512],
                            xt_d[t, c, :, :, h * 512:(h + 1) * 512]
                            .rearrange("k p n -> p k n"))
                    pair[t] = xt
                xts_c[c] = pair

            def xsq(c, t):
                # |x|^2 totals: ACT Square with free-axis accumulate, one
                # d-chunk per op (summed with the finals)
                xv = xts_c[c][t][:].bitcast(FP32)
                for k in range(2):
                    tr = tr_p.tile([128, NPC], BF16, tag="tr")
                    nc.scalar.activation(
                        tr[:], xv[:, k, :], AF.Square,
                        accum_out=pxs[:, k * 16 + t * CPC + c:
                                      k * 16 + t * CPC + c + 1],
                    )

            prep_sqs = {}   # (c, dr) -> [sq0, sq1]

            def front_units(c, dr):
                """Transform matmuls + yt ACT drains + Pool squares for
                (c, dr); runs two dir-blocks ahead of its pairwise."""
                src_t = xts_c[c][1 - dr]   # dir0: y from p2; dir1: y from p1
                yt = yt_p.tile([128, 2, NPC], FP32R, tag=f"yt{dr}", name=f"yt{c}_{dr}")
                yts[(c, dr)] = yt
                sqs = [None, None]
                prep_sqs[(c, dr)] = sqs
                pstfs = [None, None]

                def trans_mm(dcp, dc):
                    if dc == 0:
                        pstfs[dcp] = psm_p.tile([128, NPC], FP32, tag="m",
                                                name=f"pstf{c}_{dr}_{dcp}")
                    pstf = pstfs[dcp]
                    for ih in range(2):
                        nc.tensor.matmul(
                            pstf[:, ih * 512:(ih + 1) * 512],
                            mats[:, dr, dc, dcp * 128:(dcp + 1) * 128],
                            src_t[:, dc, ih * 512:(ih + 1) * 512],
                            start=(dc == 0), stop=(dc == 1),
                        )

                def trans_act(dcp):
                    bias_ap = biases[:, dr, dcp:dcp + 1]
                    sq = sq_p.tile([128, NPC], FP32R, tag="sq", name=f"sq{c}_{dr}_{dcp}")
                    nc.scalar.activation(
                        sq[:], pstfs[dcp][:], AF.Square, bias=bias_ap, scale=1.0)
                    sqs[dcp] = sq
                    nc.scalar.activation(
                        yt[:, dcp, :], pstfs[dcp][:], AF.Identity,
                        bias=bias_ap, scale=1.0)

                return [lambda: trans_mm(0, 0),
                        lambda: (trans_mm(0, 1), trans_act(0)),
                        lambda: trans_mm(1, 0),
                        lambda: (trans_mm(1, 1), trans_act(1))]

            def back_units(c, dr):
                """|y'|^2 row (psum from the G rotation) + ysb copy + |x|^2;
                runs one dir-block ahead of its pairwise."""
                sqs = prep_sqs[(c, dr)]
                psys = [None]

                def ysrow_mm(jh):
                    if jh == 0:
                        psys[0] = psm_p.tile([128, NPC], FP32, tag="m",
                                             name=f"psy{c}_{dr}")
                    for dcp in range(2):
                        nc.tensor.matmul(
                            psys[0][:, jh * 512:(jh + 1) * 512],
                            quarters,
                            sqs[dcp][:, jh * 512:(jh + 1) * 512],
                            start=(dcp == 0), stop=(dcp == 1),
                        )

                def ysb_copy():
                    ysb = ysb_p.tile([128, NPC], FP32, tag="ysb", name=f"ysb{c}_{dr}")
                    nc.scalar.copy(ysb[:], psys[0][:])
                    ysbs[(c, dr)] = ysb

                return [lambda: ysrow_mm(0),
                        lambda: (ysrow_mm(1), ysb_copy()),
                        lambda: xsq(c, dr)]

            def emit_block(pair, prepu, dma_c, slots):
                """One dir-block: 8 pairwise i-tile units with prep units
                interleaved at the given slots."""
                if dma_c is not None:
                    dma_class(dma_c)
                if pair is None:
                    for u in prepu:
                        u()
                    return
                c, dr = pair
                xside = xts_c[c][dr]      # dir0: x = p1; dir1: x = p2
                yt = yts.pop((c, dr))
                ysb = ysbs.pop((c, dr))
                sched = {}
                for s, u in zip(slots, prepu):
                    sched.setdefault(s, []).append(u)
                for it in range(IT):
                    for u in sched.pop(it, ()):
                        u()
                    col = dr * 64 + c * 8 + it
                    pg = psg_p.tile([128, NPC], FP32, tag="g", name=f"pg{col}")
                    for dc in range(2):
                        for jh in range(2):
                            nc.tensor.matmul(
                                pg[:, jh * 512:(jh + 1) * 512],
                                xside[:, dc, it * 128:(it + 1) * 128],
                                yt[:, dc, jh * 512:(jh + 1) * 512],
                                start=(dc == 0), stop=(dc == 1),
                            )
                    nc.vector._custom_dve(
                        addmin, out=scratch[:], in0=pg[:], in1=ysb[:],
                        s1=3.0e38, accum_out=pmin[:, col:col + 1],
                    )
                for us in sched.values():
                    for u in us:
                        u()

            # ---- pipeline ----
            # block b = (b//2, b%2); FRONT(b+1) in slots 1-4 of block b,
            # BACK(b+1) in slots 5-7; class DMAs one full class ahead.
            dma_class(0)
            for u in front_units(0, 0):
                u()
            for u in back_units(0, 0):
                u()
            for b in range(16):
                c, dr = b // 2, b % 2
                if dr == 0 and c + 1 < CPC:
                    dma_class(c + 1)
                units = []
                slots = []
                if b + 1 <= 15:
                    units += front_units((b + 1) // 2, (b + 1) % 2)
                    slots += [1, 2, 3, 4]
                    units += back_units((b + 1) // 2, (b + 1) % 2)
                    slots += [5, 6, 7]
                emit_block((c, dr), units, None, tuple(slots))

            # ---- finals ----
            red_min = const.tile([128, 16], FP32)
            nc.vector.tensor_reduce(
                out=red_min[:], in_=pmin[:].rearrange("p (g k) -> p g k", k=IT),
                axis=AX.X, op=ALU.add)
            red0 = const.tile([128, 16], FP32)
            nc.vector.tensor_tensor(
                out=red0[:], in0=red_min[:], in1=pxs[:, 0:16], op=ALU.add)
            red = const.tile([128, 16], FP32R)
            nc.vector.tensor_tensor(
                out=red[:], in0=red0[:], in1=pxs[:, 16:32], op=ALU.add)
            psf = psm_p.tile([1, 16], FP32, tag="m")
            nc.tensor.matmul(psf[:], ones1r, red[:], start=True, stop=True)
            outrow = const.tile([1, 16], FP32)
            nc.scalar.copy(outrow[:], psf[:])
            nc.sync.dma_start(out_d[:], outrow[:])

    nc.compile()
    return nc


def _get_nc():
    if "nc" not in _CACHE:
        _CACHE["nc"] = _build_bass()
    return _CACHE["nc"]


def kernel(protos1, protos2, W, b, num_classes):
    from concourse.bass_utils import run_bass_kernel_spmd

    nc_classes = int(num_classes)
    assert nc_classes == C and protos1.shape == (P, D)

    protos1 = np.ascontiguousarray(protos1, dtype=np.float32)
    protos2 = np.ascontiguousarray(protos2, dtype=np.float32)
    W = np.asarray(W, dtype=np.float32)
    b = np.asarray(b, dtype=np.float32)

    # host-side tiny prep: inverse + scaled transform matrices
    V = np.linalg.inv(W.T.astype(np.float64)).astype(np.float32)  # (p2-b)@V
    V2 = (-2.0 * V).astype(np.float32)                 # lhsT [d, d'] dir0
    Wt2 = (-2.0 * W.T).astype(np.float32)              # lhsT [d, d'] dir1
    bias0 = (2.0 * (b.astype(np.float64) @ V.astype(np.float64))).astype(np.float32)
    bias1 = (-2.0 * b).astype(np.float32)
    mats = np.stack([
        np.stack([V2[0:128, :], V2[128:256, :]]),
        np.stack([Wt2[0:128, :], Wt2[128:256, :]]),
    ]).astype(np.float32)                               # [2, 2, 128, 256]
    consts = np.concatenate([
        np.full((128, 128), 0.25, dtype=np.float32),
        np.full((128, 1), 1.0 / NPC, dtype=np.float32),
    ], axis=1)
    biases = np.stack([
        bias0.reshape(2, 128).T,                        # [128, 2] cols = chunk
        bias1.reshape(2, 128).T,
    ]).astype(np.float32)                               # [2, 128, 2]

    # class-major + d-major reordering: (P, D) -> (C, D, NPC) -> chunked
    full = np.stack([protos1, protos2]).reshape(2, NPC, C, D)
    xt_all = np.ascontiguousarray(full.transpose(0, 2, 3, 1))   # [2, C, D, NPC]
    xt_all = xt_all.reshape(2, C, 2, 128, NPC)

    in_maps = []
    for core in range(N_CORES):
        sl = slice(core * CPC, (core + 1) * CPC)
        in_maps.append({
            "xt": np.ascontiguousarray(xt_all[:, sl]),
            "mats": mats,
            "biases": biases,
            "consts": consts,
        })

    nc = _get_nc()
    res = run_bass_kernel_spmd(nc, in_maps, core_ids=list(range(N_CORES)))
    _CACHE["last_result"] = res

    out = np.zeros((2, C), dtype=np.float32)
    for core in range(N_CORES):
        row = res.results[core]["out"].reshape(2, CPC)
        out[:, core * CPC:(core + 1) * CPC] = row
    return out
